# revision 34
# baseline (speedup 1.0000x reference)
"""Trainium2 Bass kernel for nn_AttentionBlock (B=32, C=256, H*W=1024 tokens,
4 heads x 64 dim, out-proj + residual).

Sharding: data-parallel over batch -- 8 cores x 4 batches each.

v3 = v1's K=128 matmul structure (uniform PE tiling mode, scheduler-proof
accumulation chains) + a three-engine rebalance of everything else:

* exp SPLIT across two engines: the even head's scores go through the ACT
  engine (true exp -> bf16); the odd head's go through the DVE as a
  one-instruction Schraudolph exp in bf16-bit space:
      E = bitcast_bf16(int16(round(s * SCALE*128/ln2 + (127*128 - 7.42))))
  straight from PSUM (HW-validated round-to-nearest; end-to-end rel err
  ~5e-5 vs the 2e-2 gate, because softmax denominators cancel the error).
  This halves the former ACT bottleneck (was ~164us busy).
* q/k psum drains moved DVE->ACT (Identity activation with per-partition
  bias AP, f32r out).  un copies moved DVE->ACT.
* v bias folded into the out-proj bias (softmax rows sum to exactly 1, so
  attn_out(v + bv) = attn_out(v) + bv):  b2 = b_out + W_out^T bv, computed
  by tiny setup matmuls.  v drains become pure ACT copies.
* the reciprocal-broadcast matmul + normalize multiply are DEFERRED two
  merged-loop steps so the in-order PE queue never head-of-line blocks on
  the spread/recip/gather DMA chain; out-proj is queued from the last
  norm's completion callback.

Matmul dtype: float32r scores/proj path, bf16 PV path (E tiles bf16).
"""

import numpy as np

B_FULL = 32
N_CORES = 8
B_LOC = B_FULL // N_CORES  # 4 batches per core
C = 256
S = 1024
H = 4
D = 64
SCALE = D ** -0.5  # 0.125
P = 128
NKC = C // P  # 2 contraction chunks
NI = S // 512  # 2 i-chunks of 512
NJ = S // P  # 8 j-chunks of 128

# Schraudolph constants for bf16-bit exp of (raw_score * SCALE)
SCH_A = float(SCALE * 128.0 / np.log(2.0))
SCH_B = float(127.0 * 128.0 - 7.42)

MM_MODE = "f32r+schraudolph"

_NC_CACHE = {}


def build_nc():
    import concourse.mybir as mybir
    import concourse.tile as tile
    from concourse import bacc
    from contextlib import ExitStack

    f32 = mybir.dt.float32
    f32r = mybir.dt.float32r
    bf16 = mybir.dt.bfloat16
    i16 = mybir.dt.int16
    Exp = mybir.ActivationFunctionType.Exp
    Mult = mybir.AluOpType.mult
    Add = mybir.AluOpType.add

    nc = bacc.Bacc("TRN2")

    x_d = nc.dram_tensor("x", [B_LOC, C, S], f32, kind="ExternalInput")
    wqkv_d = nc.dram_tensor("W_qkv", [C, 3 * H * D], f32, kind="ExternalInput")
    bqkv_d = nc.dram_tensor("b_qkv", [3 * H * D], f32, kind="ExternalInput")
    wout_d = nc.dram_tensor("W_out", [C, C], f32, kind="ExternalInput")
    bout_d = nc.dram_tensor("b_out", [C], f32, kind="ExternalInput")
    out_d = nc.dram_tensor("out", [B_LOC, C, S], f32, kind="ExternalOutput")

    with ExitStack() as ctx:
        ctx.enter_context(
            nc.allow_low_precision(reason="f32r/bf16 matmul + bf16 softmax")
        )
        tc = ctx.enter_context(tile.TileContext(nc))
        const = ctx.enter_context(tc.tile_pool(name="const", bufs=1))
        xp = ctx.enter_context(tc.tile_pool(name="xp", bufs=3))
        xrp = ctx.enter_context(tc.tile_pool(name="xrp", bufs=2))

        x_sb = {}
        x_r = {}

        def load_x(b):
            t = xp.tile([P, NKC, S], f32, name=f"x_{b}", tag="x")
            nc.sync.dma_start(t, x_d[b].rearrange("(kc p) s -> p kc s", p=P))
            x_sb[b] = t

        # start the first input load before anything else queues on DMA
        # (x1 is deferred until after weight staging -- it isn't needed
        # until the second unit, and its 1MB transfer delays the weights)
        load_x(0)

        # ---- constants: DMA f32 staging, cast to f32r ----
        with tc.tile_pool(name="staging", bufs=1) as stg:
            # one contiguous burst for all of W_qkv; the (h t d) -> slot
            # shuffle happens in the f32r casts below (strided input APs)
            wq_f = stg.tile([P, NKC, 3 * H * D], f32)
            wout_f = stg.tile([P, NKC, C], f32)
            nc.gpsimd.dma_start(
                wq_f, wqkv_d.rearrange("(kc p) f -> p kc f", p=P)
            )
            nc.scalar.dma_start(wout_f, wout_d.rearrange("(kc p) n -> p kc n", p=P))

            wq_view = wq_f.rearrange(
                "p kc (hp a t d) -> p kc hp a t d", hp=2, a=2, t=3
            )
            wqk_sb = const.tile([P, NKC, 4, P], f32r)
            wv_sb = const.tile([P, NKC, H * D], f32r)
            wout_sb = const.tile([P, NKC, C], f32r)
            bv_r = const.tile([P, NKC], f32r)
            for hp in range(2):
                for t in range(2):
                    nc.vector.tensor_copy(
                        out=wqk_sb[:, :, hp * 2 + t, :].rearrange(
                            "p kc (a d) -> p kc a d", a=2
                        ),
                        in_=wq_view[:, :, hp, :, t, :],
                    )
            # the first batch's f32r cast is on the critical path to the
            # first projection matmul -- emit it before the rest of setup
            xr0 = xrp.tile([P, NKC, S], f32r, name="xr_0", tag="xr")
            nc.vector.tensor_copy(out=xr0, in_=x_sb[0])
            x_r[0] = xr0
            if B_LOC > 1:
                load_x(1)
            nc.vector.tensor_copy(
                out=wv_sb.rearrange("p kc (h2 a d) -> p kc h2 a d", h2=2, a=2),
                in_=wq_view[:, :, :, :, 2, :],
            )
            nc.vector.tensor_copy(out=wout_sb, in_=wout_f)
            # bv column: bv_r[p, kc] = bv[kc*128 + p], p = h2*64 + d
            bv_f = stg.tile([P, NKC], f32)
            bq_htd = bqkv_d.rearrange("(h t d) -> h t d", h=H, t=3)
            for kc in range(NKC):
                for h2 in range(2):
                    nc.sync.dma_start(
                        bv_f[64 * h2 : 64 * h2 + 64, kc : kc + 1],
                        bq_htd[kc * 2 + h2, 2, :, None],
                    )
            nc.vector.tensor_copy(out=bv_r, in_=bv_f)

        bqk_sb = const.tile([P, 2, 2], f32)
        bq4 = bqkv_d.rearrange("(hp a t d) -> hp a t d", hp=2, a=2, t=3)
        for hp in range(2):
            for t in range(2):
                for a in range(2):
                    nc.sync.dma_start(
                        bqk_sb[64 * a : 64 * a + 64, hp, t : t + 1],
                        bq4[hp, a, t, :, None],
                    )
        bout_sb = const.tile([P, NKC], f32)
        nc.sync.dma_start(bout_sb, bout_d.rearrange("(mc p) -> p mc", p=P))
        # dummy exp to pull the ACT table load into the setup phase
        warm = const.tile([1, 2], f32, name="warm", tag="warm")
        nc.scalar.activation(out=warm, in_=bout_sb[0:1, 0:2], func=Exp,
                             scale=0.01)

        # ones_row0: [128, 128] with row 0 = 1, rows 1-127 = 0.  K=128
        # stationary operand of "broadcast row 0 of rhs to M partitions".
        ones_row0 = const.tile([P, P], f32r)
        nc.gpsimd.memset(ones_row0.bitcast(mybir.dt.uint32), 0)
        nc.gpsimd.memset(ones_row0[0:1, :].bitcast(mybir.dt.uint32), 0x3F800000)
        ones_row64 = const.tile([P, P], f32r)
        nc.gpsimd.memset(ones_row64.bitcast(mybir.dt.uint32), 0)
        nc.gpsimd.memset(ones_row64[64:65, :].bitcast(mybir.dt.uint32), 0x3F800000)
        # persistent zero-padded rows for the reciprocal-broadcast rhs
        rcr_slots = []
        for i_ in range(3):
            t_ = const.tile([P, 512], f32r, name=f"rcr_slot{i_}", tag=f"rcrs{i_}")
            nc.gpsimd.memset(t_.bitcast(mybir.dt.uint32), 0)
            rcr_slots.append(t_)

        # ---- fold the v bias into the out-proj bias:
        #      b2[cout] = b_out[cout] + sum_d bv[d] * W_out[d, cout]
        # (transposed: out rows [1, 128], K=128 accumulation over kc) ----
        bout2_sb = const.tile([P, NKC], f32)
        bvw_sb = const.tile([P, NKC], f32)
        with tc.tile_pool(name="ps_init", bufs=2, space="PSUM") as ps_init:
            pb2 = [ps_init.tile([1, P], f32, name=f"pb2_{m}", tag=f"pb{m}")
                   for m in range(NKC)]
            for kc in range(NKC):
                for m in range(NKC):
                    nc.tensor.matmul(
                        pb2[m],
                        lhsT=bv_r[:, kc : kc + 1],
                        rhs=wout_sb[:, kc, P * m : P * (m + 1)],
                        start=(kc == 0),
                        stop=(kc == NKC - 1),
                    )
            pbs = const.tile([1, NKC * P], f32, name="pbs", tag="pbs")
            for m in range(NKC):
                nc.vector.tensor_copy(out=pbs[:, m * P : (m + 1) * P], in_=pb2[m])
            for m in range(NKC):
                nc.gpsimd.dma_start(
                    bvw_sb[:, m : m + 1], pbs[:, m * P : (m + 1) * P]
                )
            nc.vector.tensor_add(out=bout2_sb, in0=bvw_sb, in1=bout_sb)

        qkp = ctx.enter_context(tc.tile_pool(name="qkp", bufs=14))
        vp = ctx.enter_context(tc.tile_pool(name="vp", bufs=16))
        ep = ctx.enter_context(tc.tile_pool(name="ep", bufs=24))
        ap_ = ctx.enter_context(tc.tile_pool(name="ap", bufs=4))
        ahp = ctx.enter_context(tc.tile_pool(name="ahp", bufs=1))
        unp = ctx.enter_context(tc.tile_pool(name="unp", bufs=5))
        dspp = ctx.enter_context(tc.tile_pool(name="dspp", bufs=3))
        rrp = ctx.enter_context(tc.tile_pool(name="rrp", bufs=3))
        yp = ctx.enter_context(tc.tile_pool(name="yp", bufs=1))
        ps_s = ctx.enter_context(tc.tile_pool(name="ps_s", bufs=3, space="PSUM"))
        ps_pv = ctx.enter_context(tc.tile_pool(name="ps_pv", bufs=3, space="PSUM"))
        # bufs=2 so consecutive projection groups overlap their psum drains
        ps_p = ctx.enter_context(tc.tile_pool(name="ps_p", bufs=2, space="PSUM"))

        def mm64(out, lhsT, rhs, start, stop):
            # every matmul runs K=128 at base partition 0 (uniform PE tiling
            # mode; switches drain the array)
            nc.tensor.matmul(out, lhsT=lhsT, rhs=rhs, start=start, stop=stop)

        qkT = {}
        v_aug = {}
        attnT = {}

        # queue of deferred psum-group emitters (proj/outproj), consumed <=2
        # per merged-loop iteration so the in-order PE never head-of-line
        # blocks on a psum drain
        aux_q = []

        def drain_aux(n):
            for _ in range(min(n, len(aux_q))):
                aux_q.pop(0)()

        def queue_proj(b):
            xt = x_sb[b]
            if b not in x_r:
                xr = xrp.tile([P, NKC, S], f32r, name=f"xr_{b}", tag="xr")
                nc.vector.tensor_copy(out=xr, in_=xt)
                x_r[b] = xr
            qkT[b] = {}
            v_aug[b] = [None] * NJ
            qpad = {}
            for hp in range(2):
                qkT[b][(hp, 1)] = qkp.tile(
                    [P, S], f32r, name=f"kT_{b}_{hp}", tag="qkT"
                )
                for a in range(2):
                    h = 2 * hp + a
                    qp = qkp.tile([P, S], f32r, name=f"qpad_{b}_{h}", tag="qkT")
                    # zero the other head's half once; the zero rows make the
                    # full-K=128 scores matmul select only this head
                    nc.gpsimd.memset(
                        qp[64 * (1 - a) : 64 * (1 - a) + 64, :].bitcast(
                            mybir.dt.uint32
                        ),
                        0,
                    )
                    qpad[h] = qp
            qkT[b]["qpad"] = qpad

            def qk_group(b, hp, t, n):
                def emit():
                    pq = ps_p.tile([P, 512], f32, name="pq", tag="ps_p")
                    for kc in range(NKC):
                        mm64(
                            pq,
                            wqk_sb[:, kc, hp * 2 + t, :],
                            x_r[b][:, kc, 512 * n : 512 * (n + 1)],
                            start=(kc == 0),
                            stop=(kc == NKC - 1),
                        )
                    if t == 1:
                        nc.scalar.add(
                            qkT[b][(hp, 1)][:, 512 * n : 512 * (n + 1)],
                            pq,
                            bqk_sb[:, hp, t : t + 1],
                        )
                    else:
                        for a in range(2):
                            h = 2 * hp + a
                            nc.scalar.add(
                                qkT[b]["qpad"][h][
                                    64 * a : 64 * a + 64,
                                    512 * n : 512 * (n + 1),
                                ],
                                pq[64 * a : 64 * a + 64],
                                bqk_sb[64 * a : 64 * a + 64, hp, t : t + 1],
                            )

                return emit

            def v_group(b, t):
                def emit():
                    pv = ps_p.tile([P, 512], f32, name="pv", tag="ps_p")
                    pvv = pv[:, 0 : H * D]
                    for kc in range(NKC):
                        mm64(
                            pvv,
                            x_r[b][:, kc, P * t : P * (t + 1)],
                            wv_sb[:, kc, :],
                            start=(kc == 0),
                            stop=(kc == NKC - 1),
                        )
                    vt = vp.tile(
                        [P, H * (D + 1)], bf16, name=f"vaug_{b}_{t}", tag="vaug"
                    )
                    nc.scalar.copy(
                        out=vt.rearrange("p (h e) -> p h e", h=H)[:, :, 0:D],
                        in_=pvv.rearrange("p (h d) -> p h d", h=H),
                    )
                    nc.gpsimd.memset(
                        vt.rearrange("p (h e) -> p h e", h=H)[:, :, D : D + 1], 1.0
                    )
                    v_aug[b][t] = vt

                return emit

            for hp in range(2):
                for t in range(2):
                    for n in range(NI):
                        aux_q.append(qk_group(b, hp, t, n))
            for t in range(NJ):
                aux_q.append(v_group(b, t))

        def queue_outproj(b):
            yt = yp.tile([P, NKC, S], f32, name=f"y_{b}", tag="y")

            def out_group(mc, ic, last):
                def emit():
                    py = ps_p.tile([P, 512], f32, name="py", tag="ps_p")
                    for kc in range(NKC):
                        mm64(
                            py,
                            wout_sb[:, kc, P * mc : P * (mc + 1)],
                            attnT[b][kc][:, 512 * ic : 512 * (ic + 1)],
                            start=(kc == 0),
                            stop=(kc == NKC - 1),
                        )
                    nc.vector.scalar_tensor_tensor(
                        out=yt[:, mc, 512 * ic : 512 * (ic + 1)],
                        in0=py,
                        scalar=bout2_sb[:, mc : mc + 1],
                        in1=x_sb[b][:, mc, 512 * ic : 512 * (ic + 1)],
                        op0=Add,
                        op1=Add,
                    )
                    if last:
                        nc.sync.dma_start(
                            out_d[b].rearrange("(kc p) s -> p kc s", p=P), yt
                        )

                return emit

            # out-projection groups release the oldest x tile; they must
            # drain BEFORE queued projections of future batches, whose x load
            # is waiting for that very slot (else: scheduling deadlock)
            groups = [
                out_group(mc, ic, mc == NKC - 1 and ic == NI - 1)
                for mc in range(NKC)
                for ic in range(NI)
            ]
            aux_q[0:0] = groups

        norm_count = [0]
        norm2_q = []  # (due_step, emit_fn)

        def norm_part1(po0, po1, dst0, dst1, step_now, after=None):
            """Copy both heads' unnormalized PV psums to SBUF, start the
            reciprocal DMA chain, and defer the broadcast+multiply so the
            PE queue never waits on the DMA chain."""
            un0 = unp.tile([65, 512], f32, name="un0", tag="un")
            un1 = unp.tile([65, 512], f32, name="un1", tag="un")
            nc.scalar.copy(out=un0, in_=po0)
            nc.vector.tensor_copy(out=un1, in_=po1)
            dsp = dspp.tile([32, 32], f32, name="dsp", tag="dsp")
            nc.sync.dma_start(dsp[:, 0:16], un0[64:65, 0:512])
            nc.sync.dma_start(dsp[:, 16:32], un1[64:65, 0:512])
            rr = rrp.tile([32, 32], f32r, name="rr", tag="rr")
            nc.vector.reciprocal(out=rr, in_=dsp)
            rcr = rcr_slots[norm_count[0] % len(rcr_slots)]
            norm_count[0] += 1
            nc.sync.dma_start(rcr[0:1, 0:512], rr[:, 0:16])
            nc.sync.dma_start(rcr[64:65, 0:512], rr[:, 16:32])

            def part2():
                pb0 = ps_p.tile([P, 512], f32, name="pb0", tag="ps_p")
                pb1 = ps_p.tile([P, 512], f32, name="pb1", tag="ps_p")
                nc.tensor.matmul(
                    pb0[0:64, :], lhsT=ones_row0[:, 0:64], rhs=rcr,
                    start=True, stop=True,
                )
                nc.tensor.matmul(
                    pb1[0:64, :], lhsT=ones_row64[:, 0:64], rhs=rcr,
                    start=True, stop=True,
                )
                nc.vector.tensor_mul(out=dst0, in0=un0[0:64], in1=pb0[0:64])
                nc.vector.tensor_mul(out=dst1, in0=un1[0:64], in1=pb1[0:64])
                if after is not None:
                    after()

            norm2_q.append((step_now + 2, part2))

        def drain_norm2(step_now):
            while norm2_q and norm2_q[0][0] <= step_now:
                norm2_q.pop(0)[1]()

        gstep = [0]

        def unit(u, prev):
            """Emit head-pair unit u = (b, hp): scores+exp for its two heads
            (even head on ACT, odd head on DVE/Schraudolph), interleaved with
            the PREVIOUS unit's PV and queued projection groups."""
            b, hp = u
            qp0 = qkT[b]["qpad"][2 * hp]
            qp1 = qkT[b]["qpad"][2 * hp + 1]
            k = qkT[b][(hp, 1)]
            E0 = [None] * NJ  # even head: bf16 from ACT exp
            E1 = [None] * NJ  # odd head: int16 schraudolph from DVE
            if prev is not None:
                pb_, php_, pE0, pE1 = prev
                if php_ == 0 and pb_ not in attnT:
                    attnT[pb_] = [
                        ap_.tile([P, S], f32r, name=f"attnT_{pb_}_{kk}", tag="attnT")
                        for kk in range(NKC)
                    ]
                po = {}
                ah = {}
            for jc in range(NJ):
                gstep[0] += 1
                drain_norm2(gstep[0])
                drain_aux(4 if (b, hp) == (0, 0) else 2)
                ss = [
                    [
                        ps_s.tile([P, 512], f32, name=f"s{a}{ic}", tag="ps2")
                        for ic in range(NI)
                    ]
                    for a in range(2)
                ]
                E0[jc] = ep.tile([P, S], bf16, name=f"E0_{b}_{hp}_{jc}", tag="E")
                E1[jc] = ep.tile([P, S], i16, name=f"E1_{b}_{hp}_{jc}", tag="E")
                for a, qp in ((0, qp0), (1, qp1)):
                    for ic in range(NI):
                        nc.tensor.matmul(
                            ss[a][ic],
                            lhsT=k[:, P * jc : P * (jc + 1)],
                            rhs=qp[:, 512 * ic : 512 * (ic + 1)],
                            start=True,
                            stop=True,
                        )
                for ic in range(NI):
                    nc.scalar.activation(
                        out=E0[jc][:, 512 * ic : 512 * (ic + 1)],
                        in_=ss[0][ic], func=Exp, scale=SCALE,
                    )
                    nc.vector.tensor_scalar(
                        out=E1[jc][:, 512 * ic : 512 * (ic + 1)],
                        in0=ss[1][ic], scalar1=SCH_A, scalar2=SCH_B,
                        op0=Mult, op1=Add,
                    )
                if prev is not None:
                    # PV of prev unit: ic0 during iters 0-3, ic1 during 4-7
                    icp = jc // 4
                    for sub in range(2):
                        jj = (jc % 4) * 2 + sub
                        for a in range(2):
                            h = 2 * php_ + a
                            if jj == 0:
                                po[(a, icp)] = ps_pv.tile(
                                    [65, 512], f32, name="po", tag="po"
                                )
                            pE = pE0[jj] if a == 0 else pE1[jj].bitcast(bf16)
                            mm64(
                                po[(a, icp)],
                                v_aug[pb_][jj][:, 65 * h : 65 * h + 65],
                                pE[:, 512 * icp : 512 * (icp + 1)],
                                start=(jj == 0),
                                stop=(jj == NJ - 1),
                            )
                    if jc % 4 == 3:
                        if 1 not in ah:
                            ah[1] = ahp.tile(
                                [64, S], f32r, name=f"ah_{pb_}_{php_}", tag="ah"
                            )
                        dst0 = attnT[pb_][php_][0:64, 512 * icp : 512 * (icp + 1)]
                        dst1 = ah[1][:, 512 * icp : 512 * (icp + 1)]
                        after = None
                        if icp == NI - 1:
                            at_dst = attnT[pb_][php_][64:128, :]
                            ah_src = ah[1]
                            qout = pb_ if php_ == 1 else None

                            def after(at_dst=at_dst, ah_src=ah_src, qout=qout):
                                nc.gpsimd.dma_start(at_dst, ah_src)
                                if qout is not None:
                                    queue_outproj(qout)

                        norm_part1(
                            po[(0, icp)], po[(1, icp)], dst0, dst1,
                            gstep[0], after=after,
                        )
            return (b, hp, E0, E1)

        # ---- pipeline over head-pair units ----
        units = [(b, hp) for b in range(B_LOC) for hp in range(2)]
        queue_proj(0)  # drained inside unit (0,0)'s iterations
        prev = None
        for b, hp in units:
            if hp == 0 and b + 1 < B_LOC:
                queue_proj(b + 1)
                if b + 2 < B_LOC:
                    load_x(b + 2)
            prev = unit((b, hp), prev)
        # ---- drain: PV + norms of the last unit, then remaining aux ----
        b, hp, E0, E1 = prev
        if b not in attnT:
            attnT[b] = [
                ap_.tile([P, S], f32r, name=f"attnT_{b}_{kk}", tag="attnT")
                for kk in range(NKC)
            ]
        ah_last = ahp.tile([64, S], f32r, name="ah_last", tag="ah")
        yt_last = yp.tile([P, NKC, S], f32, name=f"y_{b}", tag="y")

        def out_group_last(mc, ic, last):
            def emit():
                py = ps_p.tile([P, 512], f32, name="py", tag="ps_p")
                for kc in range(NKC):
                    mm64(
                        py,
                        wout_sb[:, kc, P * mc : P * (mc + 1)],
                        attnT[b][kc][:, 512 * ic : 512 * (ic + 1)],
                        start=(kc == 0),
                        stop=(kc == NKC - 1),
                    )
                nc.vector.scalar_tensor_tensor(
                    out=yt_last[:, mc, 512 * ic : 512 * (ic + 1)],
                    in0=py,
                    scalar=bout2_sb[:, mc : mc + 1],
                    in1=x_sb[b][:, mc, 512 * ic : 512 * (ic + 1)],
                    op0=Add,
                    op1=Add,
                )
                if last:
                    nc.sync.dma_start(
                        out_d[b].rearrange("(kc p) s -> p kc s", p=P), yt_last
                    )

            return emit

        for icp in range(NI):
            po = [
                ps_pv.tile([65, 512], f32, name=f"poL{a}", tag="po")
                for a in range(2)
            ]
            for jj in range(NJ):
                gstep[0] += 1
                drain_norm2(gstep[0])
                drain_aux(1)
                for a in range(2):
                    h = 2 * hp + a
                    pE = E0[jj] if a == 0 else E1[jj].bitcast(bf16)
                    mm64(
                        po[a],
                        v_aug[b][jj][:, 65 * h : 65 * h + 65],
                        pE[:, 512 * icp : 512 * (icp + 1)],
                        start=(jj == 0),
                        stop=(jj == NJ - 1),
                    )
            dst0 = attnT[b][hp][0:64, 512 * icp : 512 * (icp + 1)]
            dst1 = ah_last[:, 512 * icp : 512 * (icp + 1)]

            def after_icp(icp=icp):
                nc.gpsimd.dma_start(
                    attnT[b][hp][64:128, 512 * icp : 512 * (icp + 1)],
                    ah_last[:, 512 * icp : 512 * (icp + 1)],
                )
                for mc in range(NKC):
                    aux_q.append(
                        out_group_last(mc, icp, icp == NI - 1 and mc == NKC - 1)
                    )

            # defer the broadcast+multiply ~4 tail steps (tail steps are
            # jj-grained, much shorter than main-loop steps) so the PE hides
            # the spread/recip/gather DMA chain behind the next icp's PV
            norm_part1(po[0], po[1], dst0, dst1, gstep[0] + 2, after=after_icp)
            drain_aux(2)
        drain_norm2(10 ** 9)
        drain_aux(len(aux_q))

    nc.compile()
    return nc


def _get_nc():
    if "nc" not in _NC_CACHE:
        _NC_CACHE["nc"] = build_nc()
    return _NC_CACHE["nc"]


def run_kernel(x, W_qkv, b_qkv, W_out, b_out, trace=False, **trace_kw):
    from concourse.bass_utils import run_bass_kernel_spmd

    nc = _get_nc()
    xs = np.ascontiguousarray(x, dtype=np.float32).reshape(B_FULL, C, S)
    shards = xs.reshape(N_CORES, B_LOC, C, S)
    common = {
        "W_qkv": np.ascontiguousarray(W_qkv, dtype=np.float32),
        "b_qkv": np.ascontiguousarray(b_qkv, dtype=np.float32),
        "W_out": np.ascontiguousarray(W_out, dtype=np.float32),
        "b_out": np.ascontiguousarray(b_out, dtype=np.float32),
    }
    in_maps = [{"x": np.ascontiguousarray(shards[i]), **common} for i in range(N_CORES)]
    res = run_bass_kernel_spmd(
        nc, in_maps, core_ids=list(range(N_CORES)), trace=trace, **trace_kw
    )
    out = np.stack([res.results[i]["out"] for i in range(N_CORES)])
    hw = int(round(np.sqrt(S)))
    return out.reshape(B_FULL, C, hw, hw).astype(np.float32), res


def kernel(x, W_qkv, b_qkv, W_out, b_out):
    out, _ = run_kernel(x, W_qkv, b_qkv, W_out, b_out)
    return out


# revision 35
# speedup vs baseline: 1.0239x; 1.0239x over previous
"""Trainium2 Bass kernel for nn_AttentionBlock (B=32, C=256, H*W=1024 tokens,
4 heads x 64 dim, out-proj + residual).

Sharding: data-parallel over batch -- 8 cores x 4 batches each.

v3 = v1's K=128 matmul structure (uniform PE tiling mode, scheduler-proof
accumulation chains) + a three-engine rebalance of everything else:

* exp SPLIT across two engines: the even head's scores go through the ACT
  engine (true exp -> bf16); the odd head's go through the DVE as a
  one-instruction Schraudolph exp in bf16-bit space:
      E = bitcast_bf16(int16(round(s * SCALE*128/ln2 + (127*128 - 7.42))))
  straight from PSUM (HW-validated round-to-nearest; end-to-end rel err
  ~5e-5 vs the 2e-2 gate, because softmax denominators cancel the error).
  This halves the former ACT bottleneck (was ~164us busy).
* q/k psum drains moved DVE->ACT (Identity activation with per-partition
  bias AP, f32r out).  un copies moved DVE->ACT.
* v bias folded into the out-proj bias (softmax rows sum to exactly 1, so
  attn_out(v + bv) = attn_out(v) + bv):  b2 = b_out + W_out^T bv, computed
  by tiny setup matmuls.  v drains become pure ACT copies.
* the reciprocal-broadcast matmul + normalize multiply are DEFERRED two
  merged-loop steps so the in-order PE queue never head-of-line blocks on
  the spread/recip/gather DMA chain; out-proj is queued from the last
  norm's completion callback.

Matmul dtype: float32r scores/proj path, bf16 PV path (E tiles bf16).
"""

import numpy as np

B_FULL = 32
N_CORES = 8
B_LOC = B_FULL // N_CORES  # 4 batches per core
C = 256
S = 1024
H = 4
D = 64
SCALE = D ** -0.5  # 0.125
P = 128
NKC = C // P  # 2 contraction chunks
NI = S // 512  # 2 i-chunks of 512
NJ = S // P  # 8 j-chunks of 128

# Schraudolph constants for bf16-bit exp of (raw_score * SCALE)
SCH_A = float(SCALE * 128.0 / np.log(2.0))
SCH_B = float(127.0 * 128.0 - 7.42)

MM_MODE = "f32r+schraudolph"

_NC_CACHE = {}


def build_nc():
    import concourse.mybir as mybir
    import concourse.tile as tile
    from concourse import bacc
    from contextlib import ExitStack

    f32 = mybir.dt.float32
    f32r = mybir.dt.float32r
    bf16 = mybir.dt.bfloat16
    i16 = mybir.dt.int16
    Exp = mybir.ActivationFunctionType.Exp
    Mult = mybir.AluOpType.mult
    Add = mybir.AluOpType.add

    nc = bacc.Bacc("TRN2")

    x_d = nc.dram_tensor("x", [B_LOC, C, S], f32, kind="ExternalInput")
    wqkv_d = nc.dram_tensor("W_qkv", [C, 3 * H * D], f32, kind="ExternalInput")
    bqkv_d = nc.dram_tensor("b_qkv", [3 * H * D], f32, kind="ExternalInput")
    wout_d = nc.dram_tensor("W_out", [C, C], f32, kind="ExternalInput")
    bout_d = nc.dram_tensor("b_out", [C], f32, kind="ExternalInput")
    out_d = nc.dram_tensor("out", [B_LOC, C, S], f32, kind="ExternalOutput")

    with ExitStack() as ctx:
        ctx.enter_context(
            nc.allow_low_precision(reason="f32r/bf16 matmul + bf16 softmax")
        )
        tc = ctx.enter_context(tile.TileContext(nc))
        const = ctx.enter_context(tc.tile_pool(name="const", bufs=1))
        xp = ctx.enter_context(tc.tile_pool(name="xp", bufs=3))
        xrp = ctx.enter_context(tc.tile_pool(name="xrp", bufs=2))

        x_sb = {}
        x_r = {}

        def load_x(b):
            t = xp.tile([P, NKC, S], f32, name=f"x_{b}", tag="x")
            nc.sync.dma_start(t, x_d[b].rearrange("(kc p) s -> p kc s", p=P))
            x_sb[b] = t

        # start the first input load before anything else queues on DMA
        # (x1 is deferred until after weight staging -- it isn't needed
        # until the second unit, and its 1MB transfer delays the weights)
        load_x(0)

        # ---- constants: DMA f32 staging, cast to f32r ----
        with tc.tile_pool(name="staging", bufs=1) as stg:
            # one contiguous burst for all of W_qkv; the (h t d) -> slot
            # shuffle happens in the f32r casts below (strided input APs)
            wq_f = stg.tile([P, NKC, 3 * H * D], f32)
            wout_f = stg.tile([P, NKC, C], f32)
            nc.gpsimd.dma_start(
                wq_f, wqkv_d.rearrange("(kc p) f -> p kc f", p=P)
            )
            nc.scalar.dma_start(wout_f, wout_d.rearrange("(kc p) n -> p kc n", p=P))

            wq_view = wq_f.rearrange(
                "p kc (hp a t d) -> p kc hp a t d", hp=2, a=2, t=3
            )
            wqk_sb = const.tile([P, NKC, 4, P], f32r)
            wv_sb = const.tile([P, NKC, H * D], f32r)
            wout_sb = const.tile([P, NKC, C], f32r)
            bv_r = const.tile([P, NKC], f32r)
            for hp in range(2):
                for t in range(2):
                    nc.vector.tensor_copy(
                        out=wqk_sb[:, :, hp * 2 + t, :].rearrange(
                            "p kc (a d) -> p kc a d", a=2
                        ),
                        in_=wq_view[:, :, hp, :, t, :],
                    )
            # the first batch's f32r cast is on the critical path to the
            # first projection matmul -- emit it before the rest of setup
            xr0 = xrp.tile([P, NKC, S], f32r, name="xr_0", tag="xr")
            nc.vector.tensor_copy(out=xr0, in_=x_sb[0])
            x_r[0] = xr0
            if B_LOC > 1:
                load_x(1)
            nc.vector.tensor_copy(
                out=wv_sb.rearrange("p kc (h2 a d) -> p kc h2 a d", h2=2, a=2),
                in_=wq_view[:, :, :, :, 2, :],
            )
            nc.vector.tensor_copy(out=wout_sb, in_=wout_f)
            # bv column: bv_r[p, kc] = bv[kc*128 + p], p = h2*64 + d
            bv_f = stg.tile([P, NKC], f32)
            bq_htd = bqkv_d.rearrange("(h t d) -> h t d", h=H, t=3)
            for kc in range(NKC):
                for h2 in range(2):
                    nc.sync.dma_start(
                        bv_f[64 * h2 : 64 * h2 + 64, kc : kc + 1],
                        bq_htd[kc * 2 + h2, 2, :, None],
                    )
            nc.vector.tensor_copy(out=bv_r, in_=bv_f)

        bqk_sb = const.tile([P, 2, 2], f32)
        bq4 = bqkv_d.rearrange("(hp a t d) -> hp a t d", hp=2, a=2, t=3)
        for hp in range(2):
            for t in range(2):
                for a in range(2):
                    nc.sync.dma_start(
                        bqk_sb[64 * a : 64 * a + 64, hp, t : t + 1],
                        bq4[hp, a, t, :, None],
                    )
        bout_sb = const.tile([P, NKC], f32)
        nc.sync.dma_start(bout_sb, bout_d.rearrange("(mc p) -> p mc", p=P))
        # dummy exp to pull the ACT table load into the setup phase
        warm = const.tile([1, 2], f32, name="warm", tag="warm")
        nc.scalar.activation(out=warm, in_=bout_sb[0:1, 0:2], func=Exp,
                             scale=0.01)

        # ones_row0: [128, 128] with row 0 = 1, rows 1-127 = 0.  K=128
        # stationary operand of "broadcast row 0 of rhs to M partitions".
        ones_row0 = const.tile([P, P], f32r)
        nc.gpsimd.memset(ones_row0.bitcast(mybir.dt.uint32), 0)
        nc.gpsimd.memset(ones_row0[0:1, :].bitcast(mybir.dt.uint32), 0x3F800000)
        ones_row64 = const.tile([P, P], f32r)
        nc.gpsimd.memset(ones_row64.bitcast(mybir.dt.uint32), 0)
        nc.gpsimd.memset(ones_row64[64:65, :].bitcast(mybir.dt.uint32), 0x3F800000)
        # persistent zero-padded rows for the reciprocal-broadcast rhs
        rcr_slots = []
        for i_ in range(3):
            t_ = const.tile([P, 512], f32r, name=f"rcr_slot{i_}", tag=f"rcrs{i_}")
            nc.gpsimd.memset(t_.bitcast(mybir.dt.uint32), 0)
            rcr_slots.append(t_)

        # ---- fold the v bias into the out-proj bias:
        #      b2[cout] = b_out[cout] + sum_d bv[d] * W_out[d, cout]
        # (transposed: out rows [1, 128], K=128 accumulation over kc) ----
        bout2_sb = const.tile([P, NKC], f32)
        bvw_sb = const.tile([P, NKC], f32)
        with tc.tile_pool(name="ps_init", bufs=2, space="PSUM") as ps_init:
            pb2 = [ps_init.tile([1, P], f32, name=f"pb2_{m}", tag=f"pb{m}")
                   for m in range(NKC)]
            for kc in range(NKC):
                for m in range(NKC):
                    nc.tensor.matmul(
                        pb2[m],
                        lhsT=bv_r[:, kc : kc + 1],
                        rhs=wout_sb[:, kc, P * m : P * (m + 1)],
                        start=(kc == 0),
                        stop=(kc == NKC - 1),
                    )
            pbs = const.tile([1, NKC * P], f32, name="pbs", tag="pbs")
            for m in range(NKC):
                nc.vector.tensor_copy(out=pbs[:, m * P : (m + 1) * P], in_=pb2[m])
            for m in range(NKC):
                nc.gpsimd.dma_start(
                    bvw_sb[:, m : m + 1], pbs[:, m * P : (m + 1) * P]
                )
            nc.vector.tensor_add(out=bout2_sb, in0=bvw_sb, in1=bout_sb)

        qkp = ctx.enter_context(tc.tile_pool(name="qkp", bufs=14))
        vp = ctx.enter_context(tc.tile_pool(name="vp", bufs=16))
        ep = ctx.enter_context(tc.tile_pool(name="ep", bufs=24))
        ap_ = ctx.enter_context(tc.tile_pool(name="ap", bufs=4))
        ahp = ctx.enter_context(tc.tile_pool(name="ahp", bufs=1))
        unp = ctx.enter_context(tc.tile_pool(name="unp", bufs=5))
        dspp = ctx.enter_context(tc.tile_pool(name="dspp", bufs=3))
        rrp = ctx.enter_context(tc.tile_pool(name="rrp", bufs=3))
        yp = ctx.enter_context(tc.tile_pool(name="yp", bufs=1))
        ps_s = ctx.enter_context(tc.tile_pool(name="ps_s", bufs=4, space="PSUM"))
        ps_pv = ctx.enter_context(tc.tile_pool(name="ps_pv", bufs=2, space="PSUM"))
        # bufs=2 so consecutive projection groups overlap their psum drains
        ps_p = ctx.enter_context(tc.tile_pool(name="ps_p", bufs=2, space="PSUM"))

        def mm64(out, lhsT, rhs, start, stop):
            # every matmul runs K=128 at base partition 0 (uniform PE tiling
            # mode; switches drain the array)
            nc.tensor.matmul(out, lhsT=lhsT, rhs=rhs, start=start, stop=stop)

        qkT = {}
        v_aug = {}
        attnT = {}

        # queue of deferred psum-group emitters (proj/outproj), consumed <=2
        # per merged-loop iteration so the in-order PE never head-of-line
        # blocks on a psum drain
        aux_q = []

        def drain_aux(n):
            for _ in range(min(n, len(aux_q))):
                aux_q.pop(0)()

        def queue_proj(b):
            xt = x_sb[b]
            if b not in x_r:
                xr = xrp.tile([P, NKC, S], f32r, name=f"xr_{b}", tag="xr")
                nc.vector.tensor_copy(out=xr, in_=xt)
                x_r[b] = xr
            qkT[b] = {}
            v_aug[b] = [None] * NJ
            qpad = {}
            for hp in range(2):
                qkT[b][(hp, 1)] = qkp.tile(
                    [P, S], f32r, name=f"kT_{b}_{hp}", tag="qkT"
                )
                for a in range(2):
                    h = 2 * hp + a
                    qp = qkp.tile([P, S], f32r, name=f"qpad_{b}_{h}", tag="qkT")
                    # zero the other head's half once; the zero rows make the
                    # full-K=128 scores matmul select only this head
                    nc.gpsimd.memset(
                        qp[64 * (1 - a) : 64 * (1 - a) + 64, :].bitcast(
                            mybir.dt.uint32
                        ),
                        0,
                    )
                    qpad[h] = qp
            qkT[b]["qpad"] = qpad

            def qk_group(b, hp, t, n):
                def emit():
                    pq = ps_p.tile([P, 512], f32, name="pq", tag="ps_p")
                    for kc in range(NKC):
                        mm64(
                            pq,
                            wqk_sb[:, kc, hp * 2 + t, :],
                            x_r[b][:, kc, 512 * n : 512 * (n + 1)],
                            start=(kc == 0),
                            stop=(kc == NKC - 1),
                        )
                    if t == 1:
                        nc.scalar.add(
                            qkT[b][(hp, 1)][:, 512 * n : 512 * (n + 1)],
                            pq,
                            bqk_sb[:, hp, t : t + 1],
                        )
                    else:
                        for a in range(2):
                            h = 2 * hp + a
                            nc.scalar.add(
                                qkT[b]["qpad"][h][
                                    64 * a : 64 * a + 64,
                                    512 * n : 512 * (n + 1),
                                ],
                                pq[64 * a : 64 * a + 64],
                                bqk_sb[64 * a : 64 * a + 64, hp, t : t + 1],
                            )

                return emit

            def v_group(b, t):
                def emit():
                    pv = ps_p.tile([P, 512], f32, name="pv", tag="ps_p")
                    pvv = pv[:, 0 : H * D]
                    for kc in range(NKC):
                        mm64(
                            pvv,
                            x_r[b][:, kc, P * t : P * (t + 1)],
                            wv_sb[:, kc, :],
                            start=(kc == 0),
                            stop=(kc == NKC - 1),
                        )
                    vt = vp.tile(
                        [P, H * (D + 1)], bf16, name=f"vaug_{b}_{t}", tag="vaug"
                    )
                    nc.scalar.copy(
                        out=vt.rearrange("p (h e) -> p h e", h=H)[:, :, 0:D],
                        in_=pvv.rearrange("p (h d) -> p h d", h=H),
                    )
                    nc.gpsimd.memset(
                        vt.rearrange("p (h e) -> p h e", h=H)[:, :, D : D + 1], 1.0
                    )
                    v_aug[b][t] = vt

                return emit

            for hp in range(2):
                for t in range(2):
                    for n in range(NI):
                        aux_q.append(qk_group(b, hp, t, n))
            for t in range(NJ):
                aux_q.append(v_group(b, t))

        def queue_outproj(b):
            yt = yp.tile([P, NKC, S], f32, name=f"y_{b}", tag="y")

            def out_group(mc, ic, last):
                def emit():
                    py = ps_p.tile([P, 512], f32, name="py", tag="ps_p")
                    for kc in range(NKC):
                        mm64(
                            py,
                            wout_sb[:, kc, P * mc : P * (mc + 1)],
                            attnT[b][kc][:, 512 * ic : 512 * (ic + 1)],
                            start=(kc == 0),
                            stop=(kc == NKC - 1),
                        )
                    nc.vector.scalar_tensor_tensor(
                        out=yt[:, mc, 512 * ic : 512 * (ic + 1)],
                        in0=py,
                        scalar=bout2_sb[:, mc : mc + 1],
                        in1=x_sb[b][:, mc, 512 * ic : 512 * (ic + 1)],
                        op0=Add,
                        op1=Add,
                    )
                    if last:
                        nc.sync.dma_start(
                            out_d[b].rearrange("(kc p) s -> p kc s", p=P), yt
                        )

                return emit

            # out-projection groups release the oldest x tile; they must
            # drain BEFORE queued projections of future batches, whose x load
            # is waiting for that very slot (else: scheduling deadlock)
            groups = [
                out_group(mc, ic, mc == NKC - 1 and ic == NI - 1)
                for mc in range(NKC)
                for ic in range(NI)
            ]
            aux_q[0:0] = groups

        norm_count = [0]
        norm2_q = []  # (due_step, emit_fn)

        def norm_part1(po0, po1, dst0, dst1, step_now, after=None):
            """Copy both heads' unnormalized PV psums to SBUF, start the
            reciprocal DMA chain, and defer the broadcast+multiply so the
            PE queue never waits on the DMA chain."""
            un0 = unp.tile([65, 512], f32, name="un0", tag="un")
            un1 = unp.tile([65, 512], f32, name="un1", tag="un")
            nc.scalar.copy(out=un0, in_=po0)
            nc.vector.tensor_copy(out=un1, in_=po1)
            dsp = dspp.tile([32, 32], f32, name="dsp", tag="dsp")
            nc.sync.dma_start(dsp[:, 0:16], un0[64:65, 0:512])
            nc.sync.dma_start(dsp[:, 16:32], un1[64:65, 0:512])
            rr = rrp.tile([32, 32], f32r, name="rr", tag="rr")
            nc.vector.reciprocal(out=rr, in_=dsp)
            rcr = rcr_slots[norm_count[0] % len(rcr_slots)]
            norm_count[0] += 1
            nc.sync.dma_start(rcr[0:1, 0:512], rr[:, 0:16])
            nc.sync.dma_start(rcr[64:65, 0:512], rr[:, 16:32])

            def part2():
                pb0 = ps_p.tile([P, 512], f32, name="pb0", tag="ps_p")
                pb1 = ps_p.tile([P, 512], f32, name="pb1", tag="ps_p")
                nc.tensor.matmul(
                    pb0[0:64, :], lhsT=ones_row0[:, 0:64], rhs=rcr,
                    start=True, stop=True,
                )
                nc.tensor.matmul(
                    pb1[0:64, :], lhsT=ones_row64[:, 0:64], rhs=rcr,
                    start=True, stop=True,
                )
                nc.vector.tensor_mul(out=dst0, in0=un0[0:64], in1=pb0[0:64])
                nc.vector.tensor_mul(out=dst1, in0=un1[0:64], in1=pb1[0:64])
                if after is not None:
                    after()

            norm2_q.append((step_now + 2, part2))

        def drain_norm2(step_now):
            while norm2_q and norm2_q[0][0] <= step_now:
                norm2_q.pop(0)[1]()

        gstep = [0]

        def unit(u, prev):
            """Emit head-pair unit u = (b, hp): scores+exp for its two heads
            (even head on ACT, odd head on DVE/Schraudolph), interleaved with
            the PREVIOUS unit's PV and queued projection groups."""
            b, hp = u
            qp0 = qkT[b]["qpad"][2 * hp]
            qp1 = qkT[b]["qpad"][2 * hp + 1]
            k = qkT[b][(hp, 1)]
            E0 = [None] * NJ  # even head: bf16 from ACT exp
            E1 = [None] * NJ  # odd head: int16 schraudolph from DVE
            if prev is not None:
                pb_, php_, pE0, pE1 = prev
                if php_ == 0 and pb_ not in attnT:
                    attnT[pb_] = [
                        ap_.tile([P, S], f32r, name=f"attnT_{pb_}_{kk}", tag="attnT")
                        for kk in range(NKC)
                    ]
                po = {}
                ah = {}
            for jc in range(NJ):
                gstep[0] += 1
                drain_norm2(gstep[0])
                drain_aux(4 if (b, hp) == (0, 0) else 2)
                ss = [
                    [
                        ps_s.tile([P, 512], f32, name=f"s{a}{ic}", tag="ps2")
                        for ic in range(NI)
                    ]
                    for a in range(2)
                ]
                E0[jc] = ep.tile([P, S], bf16, name=f"E0_{b}_{hp}_{jc}", tag="E")
                E1[jc] = ep.tile([P, S], i16, name=f"E1_{b}_{hp}_{jc}", tag="E")
                for a, qp in ((0, qp0), (1, qp1)):
                    for ic in range(NI):
                        nc.tensor.matmul(
                            ss[a][ic],
                            lhsT=k[:, P * jc : P * (jc + 1)],
                            rhs=qp[:, 512 * ic : 512 * (ic + 1)],
                            start=True,
                            stop=True,
                        )
                for ic in range(NI):
                    nc.scalar.activation(
                        out=E0[jc][:, 512 * ic : 512 * (ic + 1)],
                        in_=ss[0][ic], func=Exp, scale=SCALE,
                    )
                    nc.vector.tensor_scalar(
                        out=E1[jc][:, 512 * ic : 512 * (ic + 1)],
                        in0=ss[1][ic], scalar1=SCH_A, scalar2=SCH_B,
                        op0=Mult, op1=Add,
                    )
                if prev is not None:
                    # PV of prev unit: ic0 during iters 0-3, ic1 during 4-7
                    icp = jc // 4
                    for sub in range(2):
                        jj = (jc % 4) * 2 + sub
                        for a in range(2):
                            h = 2 * php_ + a
                            if jj == 0:
                                po[(a, icp)] = ps_pv.tile(
                                    [65, 512], f32, name="po", tag="po"
                                )
                            pE = pE0[jj] if a == 0 else pE1[jj].bitcast(bf16)
                            mm64(
                                po[(a, icp)],
                                v_aug[pb_][jj][:, 65 * h : 65 * h + 65],
                                pE[:, 512 * icp : 512 * (icp + 1)],
                                start=(jj == 0),
                                stop=(jj == NJ - 1),
                            )
                    if jc % 4 == 3:
                        if 1 not in ah:
                            ah[1] = ahp.tile(
                                [64, S], f32r, name=f"ah_{pb_}_{php_}", tag="ah"
                            )
                        dst0 = attnT[pb_][php_][0:64, 512 * icp : 512 * (icp + 1)]
                        dst1 = ah[1][:, 512 * icp : 512 * (icp + 1)]
                        after = None
                        if icp == NI - 1:
                            at_dst = attnT[pb_][php_][64:128, :]
                            ah_src = ah[1]
                            qout = pb_ if php_ == 1 else None

                            def after(at_dst=at_dst, ah_src=ah_src, qout=qout):
                                nc.gpsimd.dma_start(at_dst, ah_src)
                                if qout is not None:
                                    queue_outproj(qout)

                        norm_part1(
                            po[(0, icp)], po[(1, icp)], dst0, dst1,
                            gstep[0], after=after,
                        )
            return (b, hp, E0, E1)

        # ---- pipeline over head-pair units ----
        units = [(b, hp) for b in range(B_LOC) for hp in range(2)]
        queue_proj(0)  # drained inside unit (0,0)'s iterations
        prev = None
        for b, hp in units:
            if hp == 0 and b + 1 < B_LOC:
                queue_proj(b + 1)
                if b + 2 < B_LOC:
                    load_x(b + 2)
            prev = unit((b, hp), prev)
        # ---- drain: PV + norms of the last unit, then remaining aux ----
        b, hp, E0, E1 = prev
        if b not in attnT:
            attnT[b] = [
                ap_.tile([P, S], f32r, name=f"attnT_{b}_{kk}", tag="attnT")
                for kk in range(NKC)
            ]
        ah_last = ahp.tile([64, S], f32r, name="ah_last", tag="ah")
        yt_last = yp.tile([P, NKC, S], f32, name=f"y_{b}", tag="y")

        def out_group_last(mc, ic, last):
            def emit():
                py = ps_p.tile([P, 512], f32, name="py", tag="ps_p")
                for kc in range(NKC):
                    mm64(
                        py,
                        wout_sb[:, kc, P * mc : P * (mc + 1)],
                        attnT[b][kc][:, 512 * ic : 512 * (ic + 1)],
                        start=(kc == 0),
                        stop=(kc == NKC - 1),
                    )
                nc.vector.scalar_tensor_tensor(
                    out=yt_last[:, mc, 512 * ic : 512 * (ic + 1)],
                    in0=py,
                    scalar=bout2_sb[:, mc : mc + 1],
                    in1=x_sb[b][:, mc, 512 * ic : 512 * (ic + 1)],
                    op0=Add,
                    op1=Add,
                )
                if last:
                    nc.sync.dma_start(
                        out_d[b].rearrange("(kc p) s -> p kc s", p=P), yt_last
                    )

            return emit

        for icp in range(NI):
            po = [
                ps_pv.tile([65, 512], f32, name=f"poL{a}", tag="po")
                for a in range(2)
            ]
            for jj in range(NJ):
                gstep[0] += 1
                drain_norm2(gstep[0])
                drain_aux(1)
                for a in range(2):
                    h = 2 * hp + a
                    pE = E0[jj] if a == 0 else E1[jj].bitcast(bf16)
                    mm64(
                        po[a],
                        v_aug[b][jj][:, 65 * h : 65 * h + 65],
                        pE[:, 512 * icp : 512 * (icp + 1)],
                        start=(jj == 0),
                        stop=(jj == NJ - 1),
                    )
            dst0 = attnT[b][hp][0:64, 512 * icp : 512 * (icp + 1)]
            dst1 = ah_last[:, 512 * icp : 512 * (icp + 1)]

            def after_icp(icp=icp):
                nc.gpsimd.dma_start(
                    attnT[b][hp][64:128, 512 * icp : 512 * (icp + 1)],
                    ah_last[:, 512 * icp : 512 * (icp + 1)],
                )
                for mc in range(NKC):
                    aux_q.append(
                        out_group_last(mc, icp, icp == NI - 1 and mc == NKC - 1)
                    )

            # defer the broadcast+multiply ~4 tail steps (tail steps are
            # jj-grained, much shorter than main-loop steps) so the PE hides
            # the spread/recip/gather DMA chain behind the next icp's PV
            norm_part1(po[0], po[1], dst0, dst1, gstep[0] + 2, after=after_icp)
            drain_aux(2)
        drain_norm2(10 ** 9)
        drain_aux(len(aux_q))

    nc.compile()
    return nc


def _get_nc():
    if "nc" not in _NC_CACHE:
        _NC_CACHE["nc"] = build_nc()
    return _NC_CACHE["nc"]


def run_kernel(x, W_qkv, b_qkv, W_out, b_out, trace=False, **trace_kw):
    from concourse.bass_utils import run_bass_kernel_spmd

    nc = _get_nc()
    xs = np.ascontiguousarray(x, dtype=np.float32).reshape(B_FULL, C, S)
    shards = xs.reshape(N_CORES, B_LOC, C, S)
    common = {
        "W_qkv": np.ascontiguousarray(W_qkv, dtype=np.float32),
        "b_qkv": np.ascontiguousarray(b_qkv, dtype=np.float32),
        "W_out": np.ascontiguousarray(W_out, dtype=np.float32),
        "b_out": np.ascontiguousarray(b_out, dtype=np.float32),
    }
    in_maps = [{"x": np.ascontiguousarray(shards[i]), **common} for i in range(N_CORES)]
    res = run_bass_kernel_spmd(
        nc, in_maps, core_ids=list(range(N_CORES)), trace=trace, **trace_kw
    )
    out = np.stack([res.results[i]["out"] for i in range(N_CORES)])
    hw = int(round(np.sqrt(S)))
    return out.reshape(B_FULL, C, hw, hw).astype(np.float32), res


def kernel(x, W_qkv, b_qkv, W_out, b_out):
    out, _ = run_kernel(x, W_qkv, b_qkv, W_out, b_out)
    return out


# revision 36
# speedup vs baseline: 1.0329x; 1.0088x over previous
"""Trainium2 Bass kernel for nn_AttentionBlock (B=32, C=256, H*W=1024 tokens,
4 heads x 64 dim, out-proj + residual).

Sharding: data-parallel over batch -- 8 cores x 4 batches each.

v3 = v1's K=128 matmul structure (uniform PE tiling mode, scheduler-proof
accumulation chains) + a three-engine rebalance of everything else:

* exp SPLIT across two engines: the even head's scores go through the ACT
  engine (true exp -> bf16); the odd head's go through the DVE as a
  one-instruction Schraudolph exp in bf16-bit space:
      E = bitcast_bf16(int16(round(s * SCALE*128/ln2 + (127*128 - 7.42))))
  straight from PSUM (HW-validated round-to-nearest; end-to-end rel err
  ~5e-5 vs the 2e-2 gate, because softmax denominators cancel the error).
  This halves the former ACT bottleneck (was ~164us busy).
* q/k psum drains moved DVE->ACT (Identity activation with per-partition
  bias AP, f32r out).  un copies moved DVE->ACT.
* v bias folded into the out-proj bias (softmax rows sum to exactly 1, so
  attn_out(v + bv) = attn_out(v) + bv):  b2 = b_out + W_out^T bv, computed
  by tiny setup matmuls.  v drains become pure ACT copies.
* the reciprocal-broadcast matmul + normalize multiply are DEFERRED two
  merged-loop steps so the in-order PE queue never head-of-line blocks on
  the spread/recip/gather DMA chain; out-proj is queued from the last
  norm's completion callback.

Matmul dtype: float32r scores/proj path, bf16 PV path (E tiles bf16).
"""

import numpy as np

B_FULL = 32
N_CORES = 8
B_LOC = B_FULL // N_CORES  # 4 batches per core
C = 256
S = 1024
H = 4
D = 64
SCALE = D ** -0.5  # 0.125
P = 128
NKC = C // P  # 2 contraction chunks
NI = S // 512  # 2 i-chunks of 512
NJ = S // P  # 8 j-chunks of 128

# Schraudolph constants for bf16-bit exp of (raw_score * SCALE)
SCH_A = float(SCALE * 128.0 / np.log(2.0))
SCH_B = float(127.0 * 128.0 - 7.42)

MM_MODE = "f32r+schraudolph"

_NC_CACHE = {}


def build_nc():
    import concourse.mybir as mybir
    import concourse.tile as tile
    from concourse import bacc
    from contextlib import ExitStack

    f32 = mybir.dt.float32
    f32r = mybir.dt.float32r
    bf16 = mybir.dt.bfloat16
    i16 = mybir.dt.int16
    Exp = mybir.ActivationFunctionType.Exp
    Mult = mybir.AluOpType.mult
    Add = mybir.AluOpType.add

    nc = bacc.Bacc("TRN2")

    x_d = nc.dram_tensor("x", [B_LOC, C, S], f32, kind="ExternalInput")
    wqkv_d = nc.dram_tensor("W_qkv", [C, 3 * H * D], f32, kind="ExternalInput")
    bqkv_d = nc.dram_tensor("b_qkv", [3 * H * D], f32, kind="ExternalInput")
    wout_d = nc.dram_tensor("W_out", [C, C], f32, kind="ExternalInput")
    bout_d = nc.dram_tensor("b_out", [C], f32, kind="ExternalInput")
    out_d = nc.dram_tensor("out", [B_LOC, C, S], f32, kind="ExternalOutput")

    with ExitStack() as ctx:
        ctx.enter_context(
            nc.allow_low_precision(reason="f32r/bf16 matmul + bf16 softmax")
        )
        tc = ctx.enter_context(tile.TileContext(nc))
        const = ctx.enter_context(tc.tile_pool(name="const", bufs=1))
        xp = ctx.enter_context(tc.tile_pool(name="xp", bufs=3))
        xrp = ctx.enter_context(tc.tile_pool(name="xrp", bufs=2))

        x_sb = {}
        x_r = {}

        def load_x(b):
            t = xp.tile([P, NKC, S], f32, name=f"x_{b}", tag="x")
            nc.sync.dma_start(t, x_d[b].rearrange("(kc p) s -> p kc s", p=P))
            x_sb[b] = t

        # start the first input load before anything else queues on DMA
        # (x1 is deferred until after weight staging -- it isn't needed
        # until the second unit, and its 1MB transfer delays the weights)
        load_x(0)

        # ---- constants: DMA f32 staging, cast to f32r ----
        with tc.tile_pool(name="staging", bufs=1) as stg:
            # one contiguous burst for all of W_qkv; the (h t d) -> slot
            # shuffle happens in the f32r casts below (strided input APs)
            wq_f = stg.tile([P, NKC, 3 * H * D], f32)
            wout_f = stg.tile([P, NKC, C], f32)
            nc.gpsimd.dma_start(
                wq_f, wqkv_d.rearrange("(kc p) f -> p kc f", p=P)
            )
            nc.scalar.dma_start(wout_f, wout_d.rearrange("(kc p) n -> p kc n", p=P))

            wq_view = wq_f.rearrange(
                "p kc (hp a t d) -> p kc hp a t d", hp=2, a=2, t=3
            )
            wqk_sb = const.tile([P, NKC, 4, P], f32r)
            wv_sb = const.tile([P, NKC, H * D], f32r)
            wout_sb = const.tile([P, NKC, C], f32r)
            bv_r = const.tile([P, NKC], f32r)
            for hp in range(2):
                for t in range(2):
                    nc.vector.tensor_copy(
                        out=wqk_sb[:, :, hp * 2 + t, :].rearrange(
                            "p kc (a d) -> p kc a d", a=2
                        ),
                        in_=wq_view[:, :, hp, :, t, :],
                    )
            # the first batch's f32r cast is on the critical path to the
            # first projection matmul -- emit it before the rest of setup
            xr0 = xrp.tile([P, NKC, S], f32r, name="xr_0", tag="xr")
            nc.vector.tensor_copy(out=xr0, in_=x_sb[0])
            x_r[0] = xr0
            if B_LOC > 1:
                load_x(1)
            nc.vector.tensor_copy(
                out=wv_sb.rearrange("p kc (h2 a d) -> p kc h2 a d", h2=2, a=2),
                in_=wq_view[:, :, :, :, 2, :],
            )
            nc.vector.tensor_copy(out=wout_sb, in_=wout_f)
            # bv column: bv_r[p, kc] = bv[kc*128 + p], p = h2*64 + d
            bv_f = stg.tile([P, NKC], f32)
            bq_htd = bqkv_d.rearrange("(h t d) -> h t d", h=H, t=3)
            for kc in range(NKC):
                for h2 in range(2):
                    nc.sync.dma_start(
                        bv_f[64 * h2 : 64 * h2 + 64, kc : kc + 1],
                        bq_htd[kc * 2 + h2, 2, :, None],
                    )
            nc.vector.tensor_copy(out=bv_r, in_=bv_f)

        bqk_sb = const.tile([P, 2, 2], f32)
        bq4 = bqkv_d.rearrange("(hp a t d) -> hp a t d", hp=2, a=2, t=3)
        for hp in range(2):
            for t in range(2):
                for a in range(2):
                    nc.sync.dma_start(
                        bqk_sb[64 * a : 64 * a + 64, hp, t : t + 1],
                        bq4[hp, a, t, :, None],
                    )
        bout_sb = const.tile([P, NKC], f32)
        nc.sync.dma_start(bout_sb, bout_d.rearrange("(mc p) -> p mc", p=P))
        # dummy exp to pull the ACT table load into the setup phase
        warm = const.tile([1, 2], f32, name="warm", tag="warm")
        nc.scalar.activation(out=warm, in_=bout_sb[0:1, 0:2], func=Exp,
                             scale=0.01)

        # ones_row0: [128, 128] with row 0 = 1, rows 1-127 = 0.  K=128
        # stationary operand of "broadcast row 0 of rhs to M partitions".
        ones_row0 = const.tile([P, P], f32r)
        nc.gpsimd.memset(ones_row0.bitcast(mybir.dt.uint32), 0)
        nc.gpsimd.memset(ones_row0[0:1, :].bitcast(mybir.dt.uint32), 0x3F800000)
        ones_row64 = const.tile([P, P], f32r)
        nc.gpsimd.memset(ones_row64.bitcast(mybir.dt.uint32), 0)
        nc.gpsimd.memset(ones_row64[64:65, :].bitcast(mybir.dt.uint32), 0x3F800000)
        # persistent zero-padded rows for the reciprocal-broadcast rhs
        rcr_slots = []
        for i_ in range(3):
            t_ = const.tile([P, 512], f32r, name=f"rcr_slot{i_}", tag=f"rcrs{i_}")
            nc.gpsimd.memset(t_.bitcast(mybir.dt.uint32), 0)
            rcr_slots.append(t_)

        # ---- fold the v bias into the out-proj bias:
        #      b2[cout] = b_out[cout] + sum_d bv[d] * W_out[d, cout]
        # (transposed: out rows [1, 128], K=128 accumulation over kc) ----
        bout2_sb = const.tile([P, NKC], f32)
        bvw_sb = const.tile([P, NKC], f32)
        with tc.tile_pool(name="ps_init", bufs=2, space="PSUM") as ps_init:
            pb2 = [ps_init.tile([1, P], f32, name=f"pb2_{m}", tag=f"pb{m}")
                   for m in range(NKC)]
            for kc in range(NKC):
                for m in range(NKC):
                    nc.tensor.matmul(
                        pb2[m],
                        lhsT=bv_r[:, kc : kc + 1],
                        rhs=wout_sb[:, kc, P * m : P * (m + 1)],
                        start=(kc == 0),
                        stop=(kc == NKC - 1),
                    )
            pbs = const.tile([1, NKC * P], f32, name="pbs", tag="pbs")
            for m in range(NKC):
                nc.vector.tensor_copy(out=pbs[:, m * P : (m + 1) * P], in_=pb2[m])
            for m in range(NKC):
                nc.gpsimd.dma_start(
                    bvw_sb[:, m : m + 1], pbs[:, m * P : (m + 1) * P]
                )
            nc.vector.tensor_add(out=bout2_sb, in0=bvw_sb, in1=bout_sb)

        qkp = ctx.enter_context(tc.tile_pool(name="qkp", bufs=14))
        vp = ctx.enter_context(tc.tile_pool(name="vp", bufs=16))
        ep = ctx.enter_context(tc.tile_pool(name="ep", bufs=24))
        ap_ = ctx.enter_context(tc.tile_pool(name="ap", bufs=4))
        ahp = ctx.enter_context(tc.tile_pool(name="ahp", bufs=1))
        unp = ctx.enter_context(tc.tile_pool(name="unp", bufs=5))
        dspp = ctx.enter_context(tc.tile_pool(name="dspp", bufs=3))
        rrp = ctx.enter_context(tc.tile_pool(name="rrp", bufs=3))
        yp = ctx.enter_context(tc.tile_pool(name="yp", bufs=1))
        ps_s = ctx.enter_context(tc.tile_pool(name="ps_s", bufs=4, space="PSUM"))
        ps_pv = ctx.enter_context(tc.tile_pool(name="ps_pv", bufs=2, space="PSUM"))
        # bufs=2 so consecutive projection groups overlap their psum drains
        ps_p = ctx.enter_context(tc.tile_pool(name="ps_p", bufs=2, space="PSUM"))

        def mm64(out, lhsT, rhs, start, stop):
            # every matmul runs K=128 at base partition 0 (uniform PE tiling
            # mode; switches drain the array)
            nc.tensor.matmul(out, lhsT=lhsT, rhs=rhs, start=start, stop=stop)

        qkT = {}
        v_aug = {}
        attnT = {}

        # queue of deferred psum-group emitters (proj/outproj), consumed <=2
        # per merged-loop iteration so the in-order PE never head-of-line
        # blocks on a psum drain
        aux_q = []

        def drain_aux(n):
            for _ in range(min(n, len(aux_q))):
                aux_q.pop(0)()

        def queue_proj(b):
            xt = x_sb[b]
            if b not in x_r:
                xr = xrp.tile([P, NKC, S], f32r, name=f"xr_{b}", tag="xr")
                nc.vector.tensor_copy(out=xr, in_=xt)
                x_r[b] = xr
            qkT[b] = {}
            v_aug[b] = [None] * NJ
            qpad = {}
            for hp in range(2):
                qkT[b][(hp, 1)] = qkp.tile(
                    [P, S], f32r, name=f"kT_{b}_{hp}", tag="qkT"
                )
                for a in range(2):
                    h = 2 * hp + a
                    qp = qkp.tile([P, S], f32r, name=f"qpad_{b}_{h}", tag="qkT")
                    # zero the other head's half once; the zero rows make the
                    # full-K=128 scores matmul select only this head
                    nc.gpsimd.memset(
                        qp[64 * (1 - a) : 64 * (1 - a) + 64, :].bitcast(
                            mybir.dt.uint32
                        ),
                        0,
                    )
                    qpad[h] = qp
            qkT[b]["qpad"] = qpad

            def qk_group(b, hp, t, n):
                def emit():
                    pq = ps_p.tile([P, 512], f32, name="pq", tag="ps_p")
                    for kc in range(NKC):
                        mm64(
                            pq,
                            wqk_sb[:, kc, hp * 2 + t, :],
                            x_r[b][:, kc, 512 * n : 512 * (n + 1)],
                            start=(kc == 0),
                            stop=(kc == NKC - 1),
                        )
                    if t == 1:
                        nc.scalar.add(
                            qkT[b][(hp, 1)][:, 512 * n : 512 * (n + 1)],
                            pq,
                            bqk_sb[:, hp, t : t + 1],
                        )
                    else:
                        for a in range(2):
                            h = 2 * hp + a
                            nc.scalar.add(
                                qkT[b]["qpad"][h][
                                    64 * a : 64 * a + 64,
                                    512 * n : 512 * (n + 1),
                                ],
                                pq[64 * a : 64 * a + 64],
                                bqk_sb[64 * a : 64 * a + 64, hp, t : t + 1],
                            )

                return emit

            def v_group(b, t):
                def emit():
                    pv = ps_p.tile([P, 512], f32, name="pv", tag="ps_p")
                    pvv = pv[:, 0 : H * D]
                    for kc in range(NKC):
                        mm64(
                            pvv,
                            x_r[b][:, kc, P * t : P * (t + 1)],
                            wv_sb[:, kc, :],
                            start=(kc == 0),
                            stop=(kc == NKC - 1),
                        )
                    vt = vp.tile(
                        [P, H * (D + 1)], bf16, name=f"vaug_{b}_{t}", tag="vaug"
                    )
                    nc.scalar.copy(
                        out=vt.rearrange("p (h e) -> p h e", h=H)[:, :, 0:D],
                        in_=pvv.rearrange("p (h d) -> p h d", h=H),
                    )
                    nc.gpsimd.memset(
                        vt.rearrange("p (h e) -> p h e", h=H)[:, :, D : D + 1], 1.0
                    )
                    v_aug[b][t] = vt

                return emit

            for hp in range(2):
                for t in range(2):
                    for n in range(NI):
                        aux_q.append(qk_group(b, hp, t, n))
            for t in range(NJ):
                aux_q.append(v_group(b, t))

        def queue_outproj(b):
            yt = yp.tile([P, NKC, S], f32, name=f"y_{b}", tag="y")

            def out_group(mc, ic, last):
                def emit():
                    py = ps_p.tile([P, 512], f32, name="py", tag="ps_p")
                    for kc in range(NKC):
                        mm64(
                            py,
                            wout_sb[:, kc, P * mc : P * (mc + 1)],
                            attnT[b][kc][:, 512 * ic : 512 * (ic + 1)],
                            start=(kc == 0),
                            stop=(kc == NKC - 1),
                        )
                    nc.vector.scalar_tensor_tensor(
                        out=yt[:, mc, 512 * ic : 512 * (ic + 1)],
                        in0=py,
                        scalar=bout2_sb[:, mc : mc + 1],
                        in1=x_sb[b][:, mc, 512 * ic : 512 * (ic + 1)],
                        op0=Add,
                        op1=Add,
                    )
                    if last:
                        nc.sync.dma_start(
                            out_d[b].rearrange("(kc p) s -> p kc s", p=P), yt
                        )

                return emit

            # out-projection groups release the oldest x tile; they must
            # drain BEFORE queued projections of future batches, whose x load
            # is waiting for that very slot (else: scheduling deadlock)
            groups = [
                out_group(mc, ic, mc == NKC - 1 and ic == NI - 1)
                for mc in range(NKC)
                for ic in range(NI)
            ]
            aux_q[0:0] = groups

        norm_count = [0]
        norm2_q = []  # (due_step, emit_fn)

        def norm_part1(po0, po1, dst0, dst1, step_now, after=None):
            """Copy both heads' unnormalized PV psums to SBUF, start the
            reciprocal DMA chain, and defer the broadcast+multiply so the
            PE queue never waits on the DMA chain."""
            un0 = unp.tile([65, 512], f32, name="un0", tag="un")
            un1 = unp.tile([65, 512], f32, name="un1", tag="un")
            nc.scalar.copy(out=un0, in_=po0)
            nc.vector.tensor_copy(out=un1, in_=po1)
            dsp = dspp.tile([32, 32], f32, name="dsp", tag="dsp")
            nc.sync.dma_start(dsp[:, 0:16], un0[64:65, 0:512])
            nc.sync.dma_start(dsp[:, 16:32], un1[64:65, 0:512])
            rr = rrp.tile([32, 32], f32r, name="rr", tag="rr")
            nc.vector.reciprocal(out=rr, in_=dsp)
            rcr = rcr_slots[norm_count[0] % len(rcr_slots)]
            norm_count[0] += 1
            nc.sync.dma_start(rcr[0:1, 0:512], rr[:, 0:16])
            nc.sync.dma_start(rcr[64:65, 0:512], rr[:, 16:32])

            def part2():
                pb0 = ps_p.tile([P, 512], f32, name="pb0", tag="ps_p")
                pb1 = ps_p.tile([P, 512], f32, name="pb1", tag="ps_p")
                nc.tensor.matmul(
                    pb0[0:64, :], lhsT=ones_row0[:, 0:64], rhs=rcr,
                    start=True, stop=True,
                )
                nc.tensor.matmul(
                    pb1[0:64, :], lhsT=ones_row64[:, 0:64], rhs=rcr,
                    start=True, stop=True,
                )
                nc.vector.tensor_mul(out=dst0, in0=un0[0:64], in1=pb0[0:64])
                nc.vector.tensor_mul(out=dst1, in0=un1[0:64], in1=pb1[0:64])
                if after is not None:
                    after()

            norm2_q.append((step_now + 2, part2))

        def drain_norm2(step_now):
            while norm2_q and norm2_q[0][0] <= step_now:
                norm2_q.pop(0)[1]()

        gstep = [0]

        def unit(u, prev):
            """Emit head-pair unit u = (b, hp): scores+exp for its two heads
            (even head on ACT, odd head on DVE/Schraudolph), interleaved with
            the PREVIOUS unit's PV and queued projection groups."""
            b, hp = u
            qp0 = qkT[b]["qpad"][2 * hp]
            qp1 = qkT[b]["qpad"][2 * hp + 1]
            k = qkT[b][(hp, 1)]
            E0 = [None] * NJ  # even head: bf16 from ACT exp
            E1 = [None] * NJ  # odd head: int16 schraudolph from DVE
            if prev is not None:
                pb_, php_, pE0, pE1 = prev
                if php_ == 0 and pb_ not in attnT:
                    attnT[pb_] = [
                        ap_.tile([P, S], f32r, name=f"attnT_{pb_}_{kk}", tag="attnT")
                        for kk in range(NKC)
                    ]
                po = {}
                ah = {}
            for jc in range(NJ):
                gstep[0] += 1
                drain_norm2(gstep[0])
                drain_aux(4 if (b, hp) == (0, 0) else 2)
                ss = [
                    [
                        ps_s.tile([P, 512], f32, name=f"s{a}{ic}", tag="ps2")
                        for ic in range(NI)
                    ]
                    for a in range(2)
                ]
                E0[jc] = ep.tile([P, S], bf16, name=f"E0_{b}_{hp}_{jc}", tag="E")
                E1[jc] = ep.tile([P, S], i16, name=f"E1_{b}_{hp}_{jc}", tag="E")
                for a, qp in ((0, qp0), (1, qp1)):
                    for ic in range(NI):
                        nc.tensor.matmul(
                            ss[a][ic],
                            lhsT=k[:, P * jc : P * (jc + 1)],
                            rhs=qp[:, 512 * ic : 512 * (ic + 1)],
                            start=True,
                            stop=True,
                        )
                for ic in range(NI):
                    nc.scalar.activation(
                        out=E0[jc][:, 512 * ic : 512 * (ic + 1)],
                        in_=ss[0][ic], func=Exp, scale=SCALE,
                    )
                    nc.vector.tensor_scalar(
                        out=E1[jc][:, 512 * ic : 512 * (ic + 1)],
                        in0=ss[1][ic], scalar1=SCH_A, scalar2=SCH_B,
                        op0=Mult, op1=Add,
                    )
                if prev is not None:
                    # PV of prev unit: ic0 during iters 0-3, ic1 during 4-7
                    icp = jc // 4
                    for sub in range(2):
                        jj = (jc % 4) * 2 + sub
                        for a in range(2):
                            h = 2 * php_ + a
                            if jj == 0:
                                po[(a, icp)] = ps_pv.tile(
                                    [65, 512], f32, name="po", tag="po"
                                )
                            pE = pE0[jj] if a == 0 else pE1[jj].bitcast(bf16)
                            mm64(
                                po[(a, icp)],
                                v_aug[pb_][jj][:, 65 * h : 65 * h + 65],
                                pE[:, 512 * icp : 512 * (icp + 1)],
                                start=(jj == 0),
                                stop=(jj == NJ - 1),
                            )
                    if jc % 4 == 3:
                        if 1 not in ah:
                            ah[1] = ahp.tile(
                                [64, S], f32r, name=f"ah_{pb_}_{php_}", tag="ah"
                            )
                        dst0 = attnT[pb_][php_][0:64, 512 * icp : 512 * (icp + 1)]
                        dst1 = ah[1][:, 512 * icp : 512 * (icp + 1)]
                        after = None
                        if icp == NI - 1:
                            at_dst = attnT[pb_][php_][64:128, :]
                            ah_src = ah[1]
                            qout = pb_ if php_ == 1 else None

                            def after(at_dst=at_dst, ah_src=ah_src, qout=qout):
                                nc.gpsimd.dma_start(at_dst, ah_src)
                                if qout is not None:
                                    queue_outproj(qout)

                        norm_part1(
                            po[(0, icp)], po[(1, icp)], dst0, dst1,
                            gstep[0], after=after,
                        )
            return (b, hp, E0, E1)

        # ---- pipeline over head-pair units ----
        units = [(b, hp) for b in range(B_LOC) for hp in range(2)]
        queue_proj(0)  # drained inside unit (0,0)'s iterations
        prev = None
        for b, hp in units:
            if hp == 0 and b + 1 < B_LOC:
                queue_proj(b + 1)
                if b + 2 < B_LOC:
                    load_x(b + 2)
            prev = unit((b, hp), prev)
        # ---- drain: PV + norms of the last unit, then remaining aux ----
        b, hp, E0, E1 = prev
        if b not in attnT:
            attnT[b] = [
                ap_.tile([P, S], f32r, name=f"attnT_{b}_{kk}", tag="attnT")
                for kk in range(NKC)
            ]
        ah_last = ahp.tile([64, S], f32r, name="ah_last", tag="ah")
        yt_last = yp.tile([P, NKC, S], f32, name=f"y_{b}", tag="y")

        def out_group_last(mc, ic, last):
            def emit():
                py = ps_p.tile([P, 512], f32, name="py", tag="ps_p")
                for kc in range(NKC):
                    mm64(
                        py,
                        wout_sb[:, kc, P * mc : P * (mc + 1)],
                        attnT[b][kc][:, 512 * ic : 512 * (ic + 1)],
                        start=(kc == 0),
                        stop=(kc == NKC - 1),
                    )
                nc.vector.scalar_tensor_tensor(
                    out=yt_last[:, mc, 512 * ic : 512 * (ic + 1)],
                    in0=py,
                    scalar=bout2_sb[:, mc : mc + 1],
                    in1=x_sb[b][:, mc, 512 * ic : 512 * (ic + 1)],
                    op0=Add,
                    op1=Add,
                )
                if last is not None:
                    # per-kc half store: contiguous in both SBUF and DRAM,
                    # overlaps the remaining out-proj group's compute
                    kcs = last
                    nc.sync.dma_start(
                        out_d[b].rearrange("(kc p) s -> p kc s", p=P)[
                            :, kcs : kcs + 1, :
                        ],
                        yt_last[:, kcs : kcs + 1, :],
                    )

            return emit

        for icp in range(NI):
            po = [
                ps_pv.tile([65, 512], f32, name=f"poL{a}", tag="po")
                for a in range(2)
            ]
            for jj in range(NJ):
                gstep[0] += 1
                if jj >= 4:
                    drain_norm2(gstep[0])
                    drain_aux(1)
                for a in range(2):
                    h = 2 * hp + a
                    pE = E0[jj] if a == 0 else E1[jj].bitcast(bf16)
                    mm64(
                        po[a],
                        v_aug[b][jj][:, 65 * h : 65 * h + 65],
                        pE[:, 512 * icp : 512 * (icp + 1)],
                        start=(jj == 0),
                        stop=(jj == NJ - 1),
                    )
            dst0 = attnT[b][hp][0:64, 512 * icp : 512 * (icp + 1)]
            dst1 = ah_last[:, 512 * icp : 512 * (icp + 1)]

            def after_icp(icp=icp):
                nc.gpsimd.dma_start(
                    attnT[b][hp][64:128, 512 * icp : 512 * (icp + 1)],
                    ah_last[:, 512 * icp : 512 * (icp + 1)],
                )
                for mc in range(NKC):
                    aux_q.append(
                        out_group_last(
                            mc, icp, mc if icp == NI - 1 else None
                        )
                    )

            # defer the broadcast+multiply ~4 tail steps (tail steps are
            # jj-grained, much shorter than main-loop steps) so the PE hides
            # the spread/recip/gather DMA chain behind the next icp's PV
            norm_part1(po[0], po[1], dst0, dst1, gstep[0] + 2, after=after_icp)
            drain_aux(2)
        drain_norm2(10 ** 9)
        drain_aux(len(aux_q))

    nc.compile()
    return nc


def _get_nc():
    if "nc" not in _NC_CACHE:
        _NC_CACHE["nc"] = build_nc()
    return _NC_CACHE["nc"]


def run_kernel(x, W_qkv, b_qkv, W_out, b_out, trace=False, **trace_kw):
    from concourse.bass_utils import run_bass_kernel_spmd

    nc = _get_nc()
    xs = np.ascontiguousarray(x, dtype=np.float32).reshape(B_FULL, C, S)
    shards = xs.reshape(N_CORES, B_LOC, C, S)
    common = {
        "W_qkv": np.ascontiguousarray(W_qkv, dtype=np.float32),
        "b_qkv": np.ascontiguousarray(b_qkv, dtype=np.float32),
        "W_out": np.ascontiguousarray(W_out, dtype=np.float32),
        "b_out": np.ascontiguousarray(b_out, dtype=np.float32),
    }
    in_maps = [{"x": np.ascontiguousarray(shards[i]), **common} for i in range(N_CORES)]
    res = run_bass_kernel_spmd(
        nc, in_maps, core_ids=list(range(N_CORES)), trace=trace, **trace_kw
    )
    out = np.stack([res.results[i]["out"] for i in range(N_CORES)])
    hw = int(round(np.sqrt(S)))
    return out.reshape(B_FULL, C, hw, hw).astype(np.float32), res


def kernel(x, W_qkv, b_qkv, W_out, b_out):
    out, _ = run_kernel(x, W_qkv, b_qkv, W_out, b_out)
    return out
